# revision 1
# baseline (speedup 1.0000x reference)
"""Distributed GQA attention kernel for 8 TRN2 NeuronCores.

Problem: B=2, S=2048, D=2048, H=32 heads, KVH=4 kv-heads, HD=64 (GQA),
RoPE + causal attention + output projection, fp32 inputs/outputs.

Sharding: tensor-parallel over heads. Core c owns q-heads [4c..4c+4) and
kv-head c//2 (each kv head is shared by 2 cores; its tiny K/V projection is
recomputed on both). Per core:
  1. QKV projection from the replicated, host-pre-transposed x^T (bf16) with
     the core's weight column slice packed as one [2048, 448] bf16 rhs (k duplicated so KT
     transposes land partition-replicated).
  2. RoPE in natural layout on the DVE (weight columns de-interleaved on host
     so each head is [32 reals | 32 imags]; q.k is invariant under a common
     permutation of head dims).
  3. Q,K transposed on the PE; scores are computed transposed
     (scoresT[kpos, q]) so the softmax normalizer falls out of a ones-column
     appended to V in the PV matmul.
  4. Causal flash attention in bf16, kpos chunks processed in pairs: two
     128-kpos score matmuls into one 2-bank psum, one [128,1024] exp on ACT,
     multiplicative 0/1 mask on the diagonal chunks (post-exp, bf16 DVE),
     two PV matmuls. Diagonal pairs run FIRST within each q chunk so the
     DVE mask latency hides behind the clean chunks; phase-1 transposes lag
     their RoPE by one row tile for the same reason.
  5. Normalization: fast-approx reciprocal of the sums row (DVE), broadcast
     across 64 partitions on the idle GPSIMD engine, one DVE multiply.
  6. Attention outputs staged (transposed) to DRAM in AllToAll layout; one
     AllToAll per batch so batch-0 comm overlaps batch-1 compute. at-tile
     loads issue from the gpsimd queue so no other queue ever blocks on a
     collective.
  7. Row-sharded output projection (rows [256c..256c+256) of each batch)
     against the fully-resident bf16 wo, in two phases (batch-0 rows first)
     with explicit ordering deps so the in-order PE queue never waits on the
     second AllToAll before running work that is already eligible.
Host gathers the 8 [512, 2048] row-slices into the (2, 2048, 2048) output.
"""

import os
import sys

sys.path.insert(0, "/opt/trn_rl_repo")

import ml_dtypes
import numpy as np

import concourse.bass as bass
import concourse.mybir as mybir
import concourse.tile as tile
from concourse import bacc
from concourse.bass_utils import run_bass_kernel_spmd
from concourse.masks import make_identity
from concourse.tile_rust import add_dep_helper

N_CORES = 8
B, S, D = 2, 2048, 2048
H, KVH, HD = 32, 4, 64
HPC = H // N_CORES  # 4 q heads per core
ROWS = B * S  # 4096
RPC = S // N_CORES  # 256 output rows per core per batch

F32 = mybir.dt.float32
BF16 = mybir.dt.bfloat16
EXP = mybir.ActivationFunctionType.Exp
ADD = mybir.AluOpType.add
MULT = mybir.AluOpType.mult
DIV = mybir.AluOpType.divide


def build():
    nc = bacc.Bacc("TRN2", target_bir_lowering=False, debug=False, num_devices=N_CORES)

    xt = nc.declare_dram_parameter("xt", [D, ROWS], BF16, isOutput=False)
    wqkv = nc.declare_dram_parameter("wqkv", [D, 448], BF16, isOutput=False)
    wo = nc.declare_dram_parameter("wo", [D, D], BF16, isOutput=False)
    ropec = nc.declare_dram_parameter("ropec", [S, 384], BF16, isOutput=False)
    ropes = nc.declare_dram_parameter("ropes", [S, 384], BF16, isOutput=False)
    maskm = nc.declare_dram_parameter("maskm", [128, 2048], BF16, isOutput=False)
    out = nc.declare_dram_parameter("out", [2 * RPC, D], F32, isOutput=True)

    with tile.TileContext(nc) as tc:
        with (
            tc.tile_pool(name="sb", bufs=1) as sb,
            tc.tile_pool(name="ps", bufs=1, space="PSUM") as ps,
            tc.tile_pool(name="dr", bufs=1, space="DRAM") as dr,
        ):
            # ---- constants / weights first so projection starts ASAP ----
            identf = sb.tile([128, 128], F32, tag="identf")
            make_identity(nc, identf[:])
            identb = sb.tile([128, 128], BF16, tag="identb")
            nc.vector.tensor_copy(identb[:], identf[:])
            wqkv_sb = []
            for k in range(16):
                w = sb.tile([128, 448], BF16, tag=f"wqkv{k}", name=f"wqkv_sb{k}")
                nc.sync.dma_start(out=w[:], in_=wqkv[128 * k : 128 * (k + 1), :])
                wqkv_sb.append(w)
            maskm_sb = sb.tile([128, 2048], BF16, tag="maskm")
            # rope tables fully resident in bf16 (loaded just-in-time below)
            ct_all = sb.tile([128, 16 * 384], BF16, tag="ct_all")
            st_all = sb.tile([128, 16 * 384], BF16, tag="st_all")
            # wo tiles are declared here but DMA'd one per phase-1 rowblock so
            # the 8 MB doesn't delay the startup x^T loads.
            wo_sb = [
                sb.tile([128, D], BF16, tag=f"wo{k}", name=f"wo_sb{k}")
                for k in range(16)
            ]

            # ---- persistent per-batch activation tiles ----
            qt_sb = [[None, None], [None, None]]  # [b][i]: [128, 2048] bf16
            kt_sb = [None, None]  # [b]: [128, 2048] bf16 (KT replicated 0:64/64:128)
            vones = [None, None]  # [b]: [128, 16*65] bf16 (V | ones columns)
            for b in range(B):
                for i in range(2):
                    t = sb.tile([128, S], BF16, tag=f"qt{b}{i}", name=f"qt{b}{i}")
                    qt_sb[b][i] = t
                kt_sb[b] = sb.tile([128, S], BF16, tag=f"kt{b}", name=f"kt{b}")
                v = sb.tile([128, 16 * 65], BF16, tag=f"v{b}", name=f"vones{b}")
                nc.vector.memset(v[:], 1.0)
                vones[b] = v

            # ================= phase 1: QKV projection + RoPE + transposes
            pend = []  # lagged transpose work
            for rb in range(8):  # 512-row blocks of the flattened (B*S) rows
                xts = []
                for k in range(16):
                    t = sb.tile([128, 512], BF16, tag="xt", bufs=18, name=f"xt_{rb}_{k}")
                    # first block on the (startup-idle) ACT HWDGE queue so it
                    # streams in parallel with the weight loads on Sync
                    eng = nc.scalar if rb == 0 else nc.sync
                    eng.dma_start(
                        out=t[:],
                        in_=xt[128 * k : 128 * (k + 1), 512 * rb : 512 * (rb + 1)],
                    )
                    xts.append(t)
                # pace the (phase-4) wo loads: two of its 16 row-tiles per block
                if rb == 0:
                    nc.sync.dma_start(out=maskm_sb[:], in_=maskm[:])
                for w in (2 * rb, 2 * rb + 1):
                    nc.sync.dma_start(out=wo_sb[w][:], in_=wo[128 * w : 128 * (w + 1), :])
                if rb < 4:  # rope tiles for this block's positions (b1 reuses them)
                    for kc2 in range(4 * rb, 4 * rb + 4):
                        nc.sync.dma_start(
                            out=ct_all[:, 384 * kc2 : 384 * (kc2 + 1)],
                            in_=ropec[128 * kc2 : 128 * (kc2 + 1), :],
                        )
                        nc.sync.dma_start(
                            out=st_all[:, 384 * kc2 : 384 * (kc2 + 1)],
                            in_=ropes[128 * kc2 : 128 * (kc2 + 1), :],
                        )
                for rt in range(4):
                    r = 4 * rb + rt  # global 128-row tile index (0..31)
                    b = r // 16
                    kc = r % 16  # position tile within the batch
                    pq = ps.tile([128, 448], F32, tag="pq", bufs=2, name=f"pq_{r}")
                    for k in range(16):
                        nc.tensor.matmul(
                            pq[:],
                            xts[k][:, 128 * rt : 128 * (rt + 1)],
                            wqkv_sb[k][:],
                            start=(k == 0),
                            stop=(k == 15),
                        )
                    # single psum read frees the pq slot in ~0.6us; RoPE and
                    # the V copy then work from SBUF (bf16 fast modes)
                    pqc = sb.tile([128, 448], BF16, tag="pqc", bufs=2, name=f"pc_{r}")
                    nc.vector.tensor_copy(pqc[:], pq[:])
                    # RoPE over q + the two k replicas (6 head-blocks of [32r|32i])
                    ct = ct_all[:, 384 * kc : 384 * (kc + 1)]
                    st = st_all[:, 384 * kc : 384 * (kc + 1)]
                    tmp1 = sb.tile([128, 384], BF16, tag="tmp1", bufs=2, name=f"t1_{r}")
                    tmp2 = sb.tile([128, 384], BF16, tag="tmp2", bufs=2, name=f"t2_{r}")
                    qk = sb.tile([128, 384], BF16, tag="qk", bufs=4, name=f"qk_{r}")
                    nc.vector.tensor_tensor(tmp1[:], pqc[:, 0:384], ct, op=MULT)
                    pqv = pqc[:, 0:384].rearrange("p (h s j) -> p h s j", s=2, j=32)
                    t2v = tmp2[:].rearrange("p (h s j) -> p h s j", s=2, j=32)
                    stv = st.rearrange("p (h s j) -> p h s j", s=2, j=32)
                    # out real-half = q_imag * (-sin); out imag-half = q_real * (+sin)
                    nc.vector.tensor_tensor(
                        t2v[:, :, 0, :], pqv[:, :, 1, :], stv[:, :, 0, :], op=MULT
                    )
                    nc.vector.tensor_tensor(
                        t2v[:, :, 1, :], pqv[:, :, 0, :], stv[:, :, 1, :], op=MULT
                    )
                    nc.vector.tensor_tensor(qk[:], tmp1[:], tmp2[:], op=ADD)
                    # V -> bf16 into the ones-padded PV weights
                    nc.scalar.copy(vones[b][:, 65 * kc : 65 * kc + 64], pqc[:, 384:448])
                    # PE transposes, lagged one rowtile so the RoPE chain has
                    # a full projection's lead time (k is duplicated in the
                    # projection so KT lands replicated in one shot)
                    pend.append((qk, b, kc, r))
                    todo = [pend.pop(0)] if len(pend) > 1 else []
                    if r == 31:
                        todo += [pend.pop(0)]
                    for tqk, tb, tkc, tr in todo:
                        for i in range(2):
                            tp = ps.tile(
                                [128, 128], BF16, tag="pq", bufs=2, name=f"tp_{tr}_{i}"
                            )
                            nc.tensor.transpose(
                                tp[:], tqk[:, 128 * i : 128 * (i + 1)], identb[:]
                            )
                            nc.vector.tensor_copy(
                                qt_sb[tb][i][:, 128 * tkc : 128 * (tkc + 1)], tp[:]
                            )
                        tpk = ps.tile([128, 128], BF16, tag="pq", bufs=2, name=f"tpk_{tr}")
                        nc.tensor.transpose(tpk[:], tqk[:, 256:384], identb[:])
                        nc.vector.tensor_copy(
                            kt_sb[tb][:, 128 * tkc : 128 * (tkc + 1)], tpk[:]
                        )

            # ================= phases 2+3: attention per batch, then AllToAll
            # run attention strictly after phase 1 (the overlap costs more in
            # in-order-queue stalls than it saves)
            a2a_out = [None, None]
            last_pv = None  # ordering handle for the output projection
            first_attn = [True]
            for b in range(B):
                a2a_in = dr.tile([2048, RPC], BF16, tag=f"a2ai{b}", name=f"a2a_in{b}")
                a2a_out[b] = dr.tile([2048, RPC], BF16, tag=f"a2ao{b}", name=f"a2a_out{b}")
                for h in range(HPC):
                    qrow = 64 * (h % 2)
                    qtile = qt_sb[b][h // 2]
                    for qc in range(4):  # 512-wide q chunks
                        np2 = 2 * (qc + 1)  # causal: pairs of 128-kpos chunks
                        ot = ps.tile([65, 512], F32, tag="ot", bufs=2, name=f"ot_{b}_{h}_{qc}")
                        for p in reversed(range(np2)):
                            sp = ps.tile(
                                [128, 1024], F32, tag="s", bufs=2, name=f"s_{b}_{h}_{qc}_{p}"
                            )
                            for half in range(2):
                                kch = 2 * p + half
                                smm = nc.tensor.matmul(
                                    sp[:, 512 * half : 512 * (half + 1)],
                                    kt_sb[b][qrow : qrow + 64, 128 * kch : 128 * (kch + 1)],
                                    qtile[qrow : qrow + 64, 512 * qc : 512 * (qc + 1)],
                                    start=True,
                                    stop=True,
                                )
                            pt = sb.tile(
                                [128, 1024], BF16, tag="pt", bufs=6, name=f"pt_{b}_{h}_{qc}_{p}"
                            )
                            nc.scalar.activation(pt[:], sp[:], EXP, scale=0.125)
                            dd = p - 2 * qc
                            if dd >= 0:  # diagonal pair: multiplicative 0/1 mask
                                nc.vector.tensor_tensor(
                                    pt[:], pt[:], maskm_sb[:, 1024 * dd : 1024 * (dd + 1)],
                                    op=MULT,
                                )
                            for half in range(2):
                                kch = 2 * p + half
                                mm = nc.tensor.matmul(
                                    ot[:],
                                    vones[b][:, 65 * kch : 65 * kch + 65],
                                    pt[:, 512 * half : 512 * (half + 1)],
                                    start=(kch == 2 * np2 - 2 and half == 0),
                                    stop=(kch == 1),
                                )
                                last_pv = mm
                        # normalize: rows 0:64 are V^T P, row 64 is the softmax sum
                        sums = sb.tile([1, 512], F32, tag="sums", bufs=2, name=f"sm_{b}_{h}_{qc}")
                        nc.vector.tensor_copy(sums[:], ot[64:65, :])
                        inv = sb.tile([1, 512], F32, tag="inv", bufs=2, name=f"iv_{b}_{h}_{qc}")
                        nc.vector.reciprocal_approx_fast(inv[:], sums[:])
                        bcast = sb.tile([64, 512], F32, tag="bcast", bufs=2, name=f"bc_{b}_{h}_{qc}")
                        nc.gpsimd.partition_broadcast(bcast[:], inv[:])
                        osb = sb.tile([64, 512], BF16, tag="osb", bufs=3, name=f"o_{b}_{h}_{qc}")
                        nc.vector.tensor_tensor(osb[:], ot[0:64, :], bcast[:], op=MULT)
                        # stage into AllToAll layout: dest j rows 256j..256j+256
                        for half in range(2):
                            j = 2 * qc + half
                            nc.sync.dma_start(
                                out=a2a_in[256 * j + 64 * h : 256 * j + 64 * (h + 1), :],
                                in_=osb[:, 256 * half : 256 * (half + 1)],
                            )
                nc.gpsimd.collective_compute(
                    "AllToAll",
                    mybir.AluOpType.bypass,
                    replica_groups=[list(range(N_CORES))],
                    ins=[a2a_in[:].opt()],
                    outs=[a2a_out[b][:].opt()],
                )
                if b == 0:
                    # at-tile batch-0 halves load right after the first AllToAll
                    # (gpsimd queue, so Sync/PE never block on the collective)
                    ats = []
                    for k in range(16):
                        t = sb.tile([128, 512], BF16, tag=f"at{k}", name=f"at_{k}")
                        nc.gpsimd.dma_start(
                            out=t[:, 0:256],
                            in_=a2a_out[0][128 * k : 128 * (k + 1), :],
                        )
                        ats.append(t)
            for k in range(16):
                # split across gpsimd + scalar queues (ACT is idle by now) to
                # halve the post-AllToAll issue tail that gates oproj phase B
                eng = nc.gpsimd if k % 2 else nc.scalar
                eng.dma_start(
                    out=ats[k][:, 256:512],
                    in_=a2a_out[1][128 * k : 128 * (k + 1), :],
                )

            # ================= phase 4: output projection (my 512 rows @ wo)
            # phase A: batch-0 rows (need only AllToAll #1); phase B: batch-1.
            # Explicit deps pin the in-order PE queue to [attn b1][A][B].
            prev_phase_last = last_pv
            for rows in ([0, 1], [2, 3]):
                phase_last = None
                for n in range(4):
                    for row in rows:
                        op = ps.tile([128, 512], F32, tag="pq", bufs=2, name=f"op_{n}_{row}")
                        for k in range(16):
                            mm = nc.tensor.matmul(
                                op[:],
                                ats[k][:, 128 * row : 128 * (row + 1)],
                                wo_sb[k][:, 512 * n : 512 * (n + 1)],
                                start=(k == 0),
                                stop=(k == 15),
                            )
                            if k == 0 and prev_phase_last is not None:
                                add_dep_helper(
                                    mm.ins,
                                    prev_phase_last.ins,
                                    sync=False,
                                    reason="pin oproj phase order in PE queue",
                                )
                            phase_last = mm
                        ob = sb.tile([128, 512], F32, tag="outsb", bufs=2, name=f"ob_{n}_{row}")
                        nc.vector.tensor_copy(ob[:], op[:])
                        nc.sync.dma_start(
                            out=out[128 * row : 128 * (row + 1), 512 * n : 512 * (n + 1)],
                            in_=ob[:],
                        )
                prev_phase_last = phase_last

    nc.finalize()
    return nc


_NC_CACHE = None


def _get_nc():
    global _NC_CACHE
    if _NC_CACHE is None:
        _NC_CACHE = build()
    return _NC_CACHE


def _prep_inputs(x, freqs_cis, mask, wq, wk, wv, wo):
    """Host-side sharding / layout prep. Returns per-core input maps."""
    bf16 = ml_dtypes.bfloat16
    xt = np.ascontiguousarray(x.reshape(ROWS, D).T.astype(bf16))  # [D, B*S]
    cos = np.ascontiguousarray(freqs_cis[:, :, 0])  # [S, 32]
    sin = np.ascontiguousarray(freqs_cis[:, :, 1])
    c64 = np.concatenate([cos, cos], axis=1)  # [S, 64]
    s64 = np.concatenate([-sin, sin], axis=1)
    ropec = np.ascontiguousarray(np.tile(c64, (1, 6)).astype(bf16))  # [S, 384]
    ropes = np.ascontiguousarray(np.tile(s64, (1, 6)).astype(bf16))
    # diagonal-chunk 0/1 keep-masks: maskm[r, 512d + col] = keep(mask[col, 128d + r])
    maskm = np.empty((128, 2048), np.float32)
    for dd in range(4):
        maskm[:, 512 * dd : 512 * (dd + 1)] = mask[0:512, 128 * dd : 128 * (dd + 1)].T
    maskm = (maskm > -1.0).astype(bf16)  # 0 where masked (-1e9/-inf), 1 where kept
    perm = np.concatenate([np.arange(0, 64, 2), np.arange(1, 64, 2)])  # de-interleave
    wo_c = np.ascontiguousarray(wo.astype(bf16))

    in_maps = []
    for c in range(N_CORES):
        heads = range(HPC * c, HPC * (c + 1))
        kv = c // 2
        wq_c = np.concatenate([wq[:, 64 * h + perm] for h in heads], axis=1)
        wk_c = wk[:, 64 * kv + perm]
        wv_c = wv[:, 64 * kv : 64 * (kv + 1)]
        wqkv_c = np.ascontiguousarray(
            np.concatenate([wq_c, wk_c, wk_c, wv_c], axis=1).astype(bf16)
        )
        in_maps.append(
            {
                "xt": xt,
                "wqkv": wqkv_c,
                "wo": wo_c,
                "ropec": ropec,
                "ropes": ropes,
                "maskm": maskm,
            }
        )
    return in_maps


def kernel(x, freqs_cis, mask, wq, wk, wv, wo, _trace=False, _trace_kwargs=None):
    nc = _get_nc()
    in_maps = _prep_inputs(
        np.asarray(x, np.float32),
        np.asarray(freqs_cis, np.float32),
        np.asarray(mask, np.float32),
        np.asarray(wq, np.float32),
        np.asarray(wk, np.float32),
        np.asarray(wv, np.float32),
        np.asarray(wo, np.float32),
    )
    kwargs = {}
    if _trace:
        kwargs["trace"] = True
        if _trace_kwargs:
            kwargs.update(_trace_kwargs)
    res = run_bass_kernel_spmd(nc, in_maps, core_ids=list(range(N_CORES)), **kwargs)
    full = np.empty((B, S, D), np.float32)
    for c in range(N_CORES):
        oc = res.results[c]["out"]
        full[0, RPC * c : RPC * (c + 1)] = oc[0:RPC]
        full[1, RPC * c : RPC * (c + 1)] = oc[RPC : 2 * RPC]
    if _trace:
        kernel.last_results = res
    return full


if __name__ == "__main__":
    print("building...")
    nc = _get_nc()
    print("built")



# revision 4
# speedup vs baseline: 1.1229x; 1.1229x over previous
"""Distributed GQA attention kernel for 8 TRN2 NeuronCores.

Problem: B=2, S=2048, D=2048, H=32 heads, KVH=4 kv-heads, HD=64 (GQA),
RoPE + causal attention + output projection, fp32 inputs/outputs.

Sharding: tensor-parallel over heads. Core c owns q-heads [4c..4c+4) and
kv-head c//2 (each kv head is shared by 2 cores; its tiny K/V projection is
recomputed on both). Per core:
  1. QKV projection from the replicated, host-pre-transposed x^T (bf16) with
     the core's weight column slice packed as one [2048, 384] bf16 rhs
     (256 q | 64 k | 64 v).
  2. RoPE in natural layout on the DVE (weight columns de-interleaved on host
     so each head is [32 reals | 32 imags]; q.k is invariant under a common
     permutation of head dims).
  3. Q,K transposed on the PE; K's [64,128] transpose is copied to both
     partition halves of kt so either 64-row replica feeds the scores matmul.
     Scores are computed transposed (scoresT[kpos, q]) so the softmax
     normalizer falls out of ones-columns appended to V in the PV matmul.
  4. Causal flash attention in bf16, kpos chunks processed in pairs: two
     128-kpos score matmuls into one 2-bank psum, one [128,1024] exp on ACT,
     multiplicative 0/1 mask on the diagonal chunks (post-exp, bf16 DVE),
     two PV matmuls. Diagonal pairs run FIRST within each q chunk so the
     DVE mask latency hides behind the clean chunks; phase-1 transposes lag
     their RoPE by one row tile for the same reason.
  5. Normalization entirely on DVE: 32 replicated ones-columns in the PV
     weights land 32 identical sum rows in psum partitions 64:96; one 32-lane
     fast reciprocal + two 32-partition multiplies normalize the output.
     (gpsimd carries ONLY collectives + at-tile loads, so batch-1 compute
     never queues behind the batch-0 AllToAll.)
  6. Attention outputs staged (transposed) to DRAM in AllToAll layout; TWO
     half-collectives per batch (heads 0-1, then heads 2-3) so comm starts
     halfway through each batch's attention and the final collective only
     carries 0.5 MB.
  7. Row-sharded output projection (rows [256c..256c+256) of each batch)
     against the fully-resident bf16 wo, even k-tiles first (they arrive
     with the lo half-collective), with explicit ordering deps so the
     in-order PE queue never waits on a later collective before running work
     that is already eligible.
Host gathers the 8 [512, 2048] row-slices into the (2, 2048, 2048) output.
"""

import os
import sys

sys.path.insert(0, "/opt/trn_rl_repo")

import ml_dtypes
import numpy as np

import concourse.bass as bass
import concourse.mybir as mybir
import concourse.tile as tile
from concourse import bacc
from concourse.bass_utils import run_bass_kernel_spmd
from concourse.masks import make_identity
from concourse.tile_rust import add_dep_helper

N_CORES = 8
B, S, D = 2, 2048, 2048
H, KVH, HD = 32, 4, 64
HPC = H // N_CORES  # 4 q heads per core
ROWS = B * S  # 4096
RPC = S // N_CORES  # 256 output rows per core per batch

F32 = mybir.dt.float32
BF16 = mybir.dt.bfloat16
EXP = mybir.ActivationFunctionType.Exp
ADD = mybir.AluOpType.add
MULT = mybir.AluOpType.mult
DIV = mybir.AluOpType.divide

QKV = 384  # 256 q | 64 k | 64 v
ROPE_W = 320  # rope applies to q + k
VB = 96  # per-chunk block in the PV weights: 64 V | 32 ones


def build():
    nc = bacc.Bacc("TRN2", target_bir_lowering=False, debug=False, num_devices=N_CORES)

    xt = nc.declare_dram_parameter("xt", [D, ROWS], BF16, isOutput=False)
    wqkv = nc.declare_dram_parameter("wqkv", [D, QKV], BF16, isOutput=False)
    wo = nc.declare_dram_parameter("wo", [D, D], BF16, isOutput=False)
    ropec = nc.declare_dram_parameter("ropec", [S, ROPE_W], BF16, isOutput=False)
    ropes = nc.declare_dram_parameter("ropes", [S, ROPE_W], BF16, isOutput=False)
    maskm = nc.declare_dram_parameter("maskm", [128, 2048], BF16, isOutput=False)
    out = nc.declare_dram_parameter("out", [2 * RPC, D], F32, isOutput=True)

    with tile.TileContext(nc) as tc:
        with (
            tc.tile_pool(name="sb", bufs=1) as sb,
            tc.tile_pool(name="ps", bufs=1, space="PSUM") as ps,
            tc.tile_pool(name="dr", bufs=1, space="DRAM") as dr,
        ):
            # ---- constants / weights first so projection starts ASAP ----
            identf = sb.tile([128, 128], F32, tag="identf")
            make_identity(nc, identf[:])
            identb = sb.tile([128, 128], BF16, tag="identb")
            nc.vector.tensor_copy(identb[:], identf[:])
            wqkv_sb = []
            for k in range(16):
                w = sb.tile([128, QKV], BF16, tag=f"wqkv{k}", name=f"wqkv_sb{k}")
                nc.sync.dma_start(out=w[:], in_=wqkv[128 * k : 128 * (k + 1), :])
                wqkv_sb.append(w)
            maskm_sb = sb.tile([128, 2048], BF16, tag="maskm")
            # rope tables fully resident in bf16 (loaded just-in-time below)
            ct_all = sb.tile([128, 16 * ROPE_W], BF16, tag="ct_all")
            st_all = sb.tile([128, 16 * ROPE_W], BF16, tag="st_all")
            # wo tiles are declared here but DMA'd one per phase-1 rowblock so
            # the 8 MB doesn't delay the startup x^T loads.
            wo_sb = [
                sb.tile([128, D], BF16, tag=f"wo{k}", name=f"wo_sb{k}")
                for k in range(16)
            ]

            # ---- persistent per-batch activation tiles ----
            qt_sb = [[None, None], [None, None]]  # [b][i]: [128, 2048] bf16
            kt_sb = [None, None]  # [b]: [128, 2048] bf16 (KT replicated 0:64/64:128)
            vones = [None, None]  # [b]: [128, 16*VB] bf16 (V | 32 ones columns)
            for b in range(B):
                for i in range(2):
                    t = sb.tile([128, S], BF16, tag=f"qt{b}{i}", name=f"qt{b}{i}")
                    qt_sb[b][i] = t
                kt_sb[b] = sb.tile([128, S], BF16, tag=f"kt{b}", name=f"kt{b}")
                v = sb.tile([128, 16 * VB], BF16, tag=f"v{b}", name=f"vones{b}")
                nc.vector.memset(v[:], 1.0)
                vones[b] = v

            # ================= phase 1: QKV projection + RoPE + transposes
            pend = []  # lagged transpose work
            for rb in range(8):  # 512-row blocks of the flattened (B*S) rows
                xts = []
                for k in range(16):
                    t = sb.tile([128, 512], BF16, tag="xt", bufs=18, name=f"xt_{rb}_{k}")
                    # first block on the (startup-idle) ACT HWDGE queue so it
                    # streams in parallel with the weight loads on Sync
                    eng = nc.scalar if rb == 0 else nc.sync
                    eng.dma_start(
                        out=t[:],
                        in_=xt[128 * k : 128 * (k + 1), 512 * rb : 512 * (rb + 1)],
                    )
                    xts.append(t)
                # pace the (phase-4) wo loads: two of its 16 row-tiles per block
                if rb == 0:
                    nc.sync.dma_start(out=maskm_sb[:], in_=maskm[:])
                for w in (2 * rb, 2 * rb + 1):
                    nc.sync.dma_start(out=wo_sb[w][:], in_=wo[128 * w : 128 * (w + 1), :])
                if rb < 4:  # rope tiles for this block's positions (b1 reuses them)
                    for kc2 in range(4 * rb, 4 * rb + 4):
                        nc.sync.dma_start(
                            out=ct_all[:, ROPE_W * kc2 : ROPE_W * (kc2 + 1)],
                            in_=ropec[128 * kc2 : 128 * (kc2 + 1), :],
                        )
                        nc.sync.dma_start(
                            out=st_all[:, ROPE_W * kc2 : ROPE_W * (kc2 + 1)],
                            in_=ropes[128 * kc2 : 128 * (kc2 + 1), :],
                        )
                for rt in range(4):
                    r = 4 * rb + rt  # global 128-row tile index (0..31)
                    b = r // 16
                    kc = r % 16  # position tile within the batch
                    pq = ps.tile([128, QKV], F32, tag="pq", bufs=2, name=f"pq_{r}")
                    for k in range(16):
                        nc.tensor.matmul(
                            pq[:],
                            xts[k][:, 128 * rt : 128 * (rt + 1)],
                            wqkv_sb[k][:],
                            start=(k == 0),
                            stop=(k == 15),
                        )
                    # single psum read frees the pq slot in ~0.6us; RoPE and
                    # the V copy then work from SBUF (bf16 fast modes)
                    pqc = sb.tile([128, QKV], BF16, tag="pqc", bufs=2, name=f"pc_{r}")
                    nc.vector.tensor_copy(pqc[:], pq[:])
                    # RoPE over q + k (5 head-blocks of [32r|32i])
                    ct = ct_all[:, ROPE_W * kc : ROPE_W * (kc + 1)]
                    st = st_all[:, ROPE_W * kc : ROPE_W * (kc + 1)]
                    tmp1 = sb.tile([128, ROPE_W], BF16, tag="tmp1", bufs=2, name=f"t1_{r}")
                    tmp2 = sb.tile([128, ROPE_W], BF16, tag="tmp2", bufs=2, name=f"t2_{r}")
                    qk = sb.tile([128, ROPE_W], BF16, tag="qk", bufs=4, name=f"qk_{r}")
                    nc.vector.tensor_tensor(tmp1[:], pqc[:, 0:ROPE_W], ct, op=MULT)
                    pqv = pqc[:, 0:ROPE_W].rearrange("p (h s j) -> p h s j", s=2, j=32)
                    t2v = tmp2[:].rearrange("p (h s j) -> p h s j", s=2, j=32)
                    stv = st.rearrange("p (h s j) -> p h s j", s=2, j=32)
                    # out real-half = q_imag * (-sin); out imag-half = q_real * (+sin)
                    nc.vector.tensor_tensor(
                        t2v[:, :, 0, :], pqv[:, :, 1, :], stv[:, :, 0, :], op=MULT
                    )
                    nc.vector.tensor_tensor(
                        t2v[:, :, 1, :], pqv[:, :, 0, :], stv[:, :, 1, :], op=MULT
                    )
                    nc.vector.tensor_tensor(qk[:], tmp1[:], tmp2[:], op=ADD)
                    # V -> bf16 into the ones-padded PV weights
                    nc.scalar.copy(vones[b][:, VB * kc : VB * kc + 64], pqc[:, 320:384])
                    # PE transposes, lagged one rowtile so the RoPE chain has
                    # a full projection's lead time
                    pend.append((qk, b, kc, r))
                    todo = [pend.pop(0)] if len(pend) > 1 else []
                    if r == 31:
                        todo += [pend.pop(0)]
                    for tqk, tb, tkc, tr in todo:
                        for i in range(2):
                            tp = ps.tile(
                                [128, 128], BF16, tag="pq", bufs=2, name=f"tp_{tr}_{i}"
                            )
                            nc.tensor.transpose(
                                tp[:], tqk[:, 128 * i : 128 * (i + 1)], identb[:]
                            )
                            nc.vector.tensor_copy(
                                qt_sb[tb][i][:, 128 * tkc : 128 * (tkc + 1)], tp[:]
                            )
                        # K: [128, 64] -> [64, 128], then copy into both
                        # partition halves of kt (scores read either replica)
                        tpk = ps.tile([64, 128], BF16, tag="pq", bufs=2, name=f"tpk_{tr}")
                        nc.tensor.transpose(tpk[:], tqk[:, 256:320], identb[:])
                        nc.vector.tensor_copy(
                            kt_sb[tb][0:64, 128 * tkc : 128 * (tkc + 1)], tpk[:]
                        )
                        nc.vector.tensor_copy(
                            kt_sb[tb][64:128, 128 * tkc : 128 * (tkc + 1)], tpk[:]
                        )

            # ================= phases 2+3: attention per batch, then AllToAll
            # run attention strictly after phase 1 (the overlap costs more in
            # in-order-queue stalls than it saves)
            a2a_out = [[None, None], [None, None]]  # [b][half]
            last_pv = None  # ordering handle for the output projection
            ats = [
                sb.tile([128, 512], BF16, tag=f"at{k}", name=f"at_{k}") for k in range(16)
            ]
            for b in range(B):
                a2a_in = [
                    dr.tile([1024, RPC], BF16, tag=f"a2ai{b}{p}", name=f"a2a_in{b}{p}")
                    for p in range(2)
                ]
                a2a_out[b] = [
                    dr.tile([1024, RPC], BF16, tag=f"a2ao{b}{p}", name=f"a2a_out{b}{p}")
                    for p in range(2)
                ]
                for h in range(HPC):
                    qrow = 64 * (h % 2)
                    qtile = qt_sb[b][h // 2]
                    for qc in range(4):  # 512-wide q chunks
                        np2 = 2 * (qc + 1)  # causal: pairs of 128-kpos chunks
                        ot = ps.tile([96, 512], F32, tag="ot", bufs=2, name=f"ot_{b}_{h}_{qc}")
                        for p in reversed(range(np2)):
                            sp = ps.tile(
                                [128, 1024], F32, tag="s", bufs=2, name=f"s_{b}_{h}_{qc}_{p}"
                            )
                            for half in range(2):
                                kch = 2 * p + half
                                smm = nc.tensor.matmul(
                                    sp[:, 512 * half : 512 * (half + 1)],
                                    kt_sb[b][qrow : qrow + 64, 128 * kch : 128 * (kch + 1)],
                                    qtile[qrow : qrow + 64, 512 * qc : 512 * (qc + 1)],
                                    start=True,
                                    stop=True,
                                )
                            pt = sb.tile(
                                [128, 1024], BF16, tag="pt", bufs=6, name=f"pt_{b}_{h}_{qc}_{p}"
                            )
                            nc.scalar.activation(pt[:], sp[:], EXP, scale=0.125)
                            dd = p - 2 * qc
                            if dd >= 0:  # diagonal pair: multiplicative 0/1 mask
                                nc.vector.tensor_tensor(
                                    pt[:], pt[:], maskm_sb[:, 1024 * dd : 1024 * (dd + 1)],
                                    op=MULT,
                                )
                            for half in range(2):
                                kch = 2 * p + half
                                mm = nc.tensor.matmul(
                                    ot[:],
                                    vones[b][:, VB * kch : VB * kch + VB],
                                    pt[:, 512 * half : 512 * (half + 1)],
                                    start=(kch == 2 * np2 - 2 and half == 0),
                                    stop=(kch == 1),
                                )
                                last_pv = mm
                        # normalize: rows 0:64 are V^T P, rows 64:96 the softmax
                        # sums (32 identical copies) -> 32-lane reciprocal + two
                        # 32-partition multiplies, all on DVE
                        sums = sb.tile([32, 512], F32, tag="sums", bufs=2, name=f"sm_{b}_{h}_{qc}")
                        nc.vector.tensor_copy(sums[:], ot[64:96, :])
                        inv = sb.tile([32, 512], F32, tag="inv", bufs=2, name=f"iv_{b}_{h}_{qc}")
                        nc.vector.reciprocal_approx_fast(inv[:], sums[:])
                        osb = sb.tile([64, 512], BF16, tag="osb", bufs=3, name=f"o_{b}_{h}_{qc}")
                        nc.vector.tensor_tensor(osb[0:32, :], ot[0:32, :], inv[:], op=MULT)
                        nc.vector.tensor_tensor(osb[32:64, :], ot[32:64, :], inv[:], op=MULT)
                        # stage into AllToAll layout: dest j rows 128j..128j+128
                        # in the lo (heads 0-1) / hi (heads 2-3) half buffer
                        abuf = a2a_in[h // 2]
                        for half in range(2):
                            j = 2 * qc + half
                            nc.sync.dma_start(
                                out=abuf[128 * j + 64 * (h % 2) : 128 * j + 64 * (h % 2 + 1), :],
                                in_=osb[:, 256 * half : 256 * (half + 1)],
                            )
                    if h % 2 == 1:  # lo half ready after h=1, hi after h=3
                        p = h // 2
                        nc.gpsimd.collective_compute(
                            "AllToAll",
                            mybir.AluOpType.bypass,
                            replica_groups=[list(range(N_CORES))],
                            ins=[a2a_in[p][:].opt()],
                            outs=[a2a_out[b][p][:].opt()],
                        )
                        # at-tile loads right after each half-collective, on the
                        # gpsimd queue (so Sync/PE never block on a collective);
                        # the last batch's hi loads split with the idle ACT
                        # queue to halve the issue tail gating oproj phase B.
                        # Even k-tiles come from lo, odd from hi.
                        for k in range(p, 16, 2):
                            eng = nc.scalar if (b == 1 and p == 1 and k % 4 == 3) else nc.gpsimd
                            eng.dma_start(
                                out=ats[k][:, 256 * b : 256 * (b + 1)],
                                in_=a2a_out[b][p][128 * (k // 2) : 128 * (k // 2) + 128, :],
                            )

            # ================= phase 4: output projection (my 512 rows @ wo)
            # phase A: batch-0 rows (need only batch-0 collectives); phase B:
            # batch-1. Even k-tiles first: they arrive with the lo half.
            # Explicit deps pin the in-order PE queue to [attn b1][A][B].
            korder = list(range(0, 16, 2)) + list(range(1, 16, 2))
            prev_phase_last = last_pv
            for rows in ([0, 1], [2, 3]):
                phase_last = None
                for n in range(4):
                    for row in rows:
                        op = ps.tile([128, 512], F32, tag="pq", bufs=2, name=f"op_{n}_{row}")
                        for ki, k in enumerate(korder):
                            mm = nc.tensor.matmul(
                                op[:],
                                ats[k][:, 128 * row : 128 * (row + 1)],
                                wo_sb[k][:, 512 * n : 512 * (n + 1)],
                                start=(ki == 0),
                                stop=(ki == 15),
                            )
                            if ki == 0 and prev_phase_last is not None:
                                add_dep_helper(
                                    mm.ins,
                                    prev_phase_last.ins,
                                    sync=False,
                                    reason="pin oproj phase order in PE queue",
                                )
                            phase_last = mm
                        ob = sb.tile([128, 512], F32, tag="outsb", bufs=2, name=f"ob_{n}_{row}")
                        nc.vector.tensor_copy(ob[:], op[:])
                        nc.sync.dma_start(
                            out=out[128 * row : 128 * (row + 1), 512 * n : 512 * (n + 1)],
                            in_=ob[:],
                        )
                prev_phase_last = phase_last

    nc.finalize()
    return nc


_NC_CACHE = None


def _get_nc():
    global _NC_CACHE
    if _NC_CACHE is None:
        _NC_CACHE = build()
    return _NC_CACHE


def _prep_inputs(x, freqs_cis, mask, wq, wk, wv, wo):
    """Host-side sharding / layout prep. Returns per-core input maps."""
    bf16 = ml_dtypes.bfloat16
    xt = np.ascontiguousarray(x.reshape(ROWS, D).T.astype(bf16))  # [D, B*S]
    cos = np.ascontiguousarray(freqs_cis[:, :, 0])  # [S, 32]
    sin = np.ascontiguousarray(freqs_cis[:, :, 1])
    c64 = np.concatenate([cos, cos], axis=1)  # [S, 64]
    s64 = np.concatenate([-sin, sin], axis=1)
    ropec = np.ascontiguousarray(np.tile(c64, (1, 5)).astype(bf16))  # [S, 320]
    ropes = np.ascontiguousarray(np.tile(s64, (1, 5)).astype(bf16))
    # diagonal-chunk 0/1 keep-masks: maskm[r, 512d + col] = keep(mask[col, 128d + r])
    maskm = np.empty((128, 2048), np.float32)
    for dd in range(4):
        maskm[:, 512 * dd : 512 * (dd + 1)] = mask[0:512, 128 * dd : 128 * (dd + 1)].T
    maskm = (maskm > -1.0).astype(bf16)  # 0 where masked (-1e9/-inf), 1 where kept
    perm = np.concatenate([np.arange(0, 64, 2), np.arange(1, 64, 2)])  # de-interleave
    wo_c = np.ascontiguousarray(wo.astype(bf16))

    in_maps = []
    for c in range(N_CORES):
        heads = range(HPC * c, HPC * (c + 1))
        kv = c // 2
        wq_c = np.concatenate([wq[:, 64 * h + perm] for h in heads], axis=1)
        wk_c = wk[:, 64 * kv + perm]
        wv_c = wv[:, 64 * kv : 64 * (kv + 1)]
        wqkv_c = np.ascontiguousarray(
            np.concatenate([wq_c, wk_c, wv_c], axis=1).astype(bf16)
        )
        in_maps.append(
            {
                "xt": xt,
                "wqkv": wqkv_c,
                "wo": wo_c,
                "ropec": ropec,
                "ropes": ropes,
                "maskm": maskm,
            }
        )
    return in_maps


def kernel(x, freqs_cis, mask, wq, wk, wv, wo, _trace=False, _trace_kwargs=None):
    nc = _get_nc()
    in_maps = _prep_inputs(
        np.asarray(x, np.float32),
        np.asarray(freqs_cis, np.float32),
        np.asarray(mask, np.float32),
        np.asarray(wq, np.float32),
        np.asarray(wk, np.float32),
        np.asarray(wv, np.float32),
        np.asarray(wo, np.float32),
    )
    kwargs = {}
    if _trace:
        kwargs["trace"] = True
        if _trace_kwargs:
            kwargs.update(_trace_kwargs)
    res = run_bass_kernel_spmd(nc, in_maps, core_ids=list(range(N_CORES)), **kwargs)
    full = np.empty((B, S, D), np.float32)
    for c in range(N_CORES):
        oc = res.results[c]["out"]
        full[0, RPC * c : RPC * (c + 1)] = oc[0:RPC]
        full[1, RPC * c : RPC * (c + 1)] = oc[RPC : 2 * RPC]
    if _trace:
        kernel.last_results = res
    return full


if __name__ == "__main__":
    print("building...")
    nc = _get_nc()
    print("built")


# revision 8
# speedup vs baseline: 1.2722x; 1.1329x over previous
"""Distributed GQA attention kernel for 8 TRN2 NeuronCores.

Problem: B=2, S=2048, D=2048, H=32 heads, KVH=4 kv-heads, HD=64 (GQA),
RoPE + causal attention + output projection, fp32 inputs/outputs.

Sharding: tensor-parallel over heads. Core c owns q-heads [4c..4c+4) and
kv-head c//2 (each kv head is shared by 2 cores; its tiny K/V projection is
recomputed on both). Per core:
  1. QKV projection from the replicated, host-pre-transposed x^T (bf16) with
     the core's weight column slice packed as one [2048, 384] bf16 rhs
     (256 q | 64 k | 64 v).
  2. RoPE in natural layout on the DVE (weight columns de-interleaved on host
     so each head is [32 reals | 32 imags]; q.k is invariant under a common
     permutation of head dims).
  3. Q,K transposed on the PE; K's [64,128] transpose is copied to both
     partition halves of kt so either 64-row replica feeds the scores matmul.
     Scores are computed transposed (scoresT[kpos, q]) so the softmax
     normalizer falls out of ones-columns appended to V in the PV matmul.
  4. Causal flash attention in bf16, kpos chunks processed in pairs: two
     128-kpos score matmuls into one 2-bank psum, one [128,1024] exp on ACT,
     multiplicative 0/1 mask on the diagonal chunks (post-exp, bf16 DVE),
     two PV matmuls. Diagonal pairs run FIRST within each q chunk so the
     DVE mask latency hides behind the clean chunks; phase-1 transposes lag
     their RoPE by one row tile for the same reason.
  5. Normalization entirely on DVE: 32 replicated ones-columns in the PV
     weights land 32 identical sum rows in psum partitions 64:96; one 32-lane
     fast reciprocal + two 32-partition multiplies normalize the output.
     (gpsimd carries ONLY collectives + at-tile loads, so batch-1 compute
     never queues behind the batch-0 AllToAll.)
  6. Attention outputs staged (transposed) to DRAM in AllToAll layout; TWO
     half-collectives per batch (heads 0-1, then heads 2-3) so comm starts
     halfway through each batch's attention and the final collective only
     carries 0.5 MB.
  7. Row-sharded output projection (rows [256c..256c+256) of each batch)
     against the fully-resident bf16 wo, even k-tiles first (they arrive
     with the lo half-collective), with explicit ordering deps so the
     in-order PE queue never waits on a later collective before running work
     that is already eligible.
Host gathers the 8 [512, 2048] row-slices into the (2, 2048, 2048) output.
"""

import os
import sys

sys.path.insert(0, "/opt/trn_rl_repo")

import ml_dtypes
import numpy as np

import concourse.bass as bass
import concourse.mybir as mybir
import concourse.tile as tile
from concourse import bacc
from concourse.bass_utils import run_bass_kernel_spmd
from concourse.masks import make_identity
from concourse.tile_rust import add_dep_helper

N_CORES = 8
B, S, D = 2, 2048, 2048
H, KVH, HD = 32, 4, 64
HPC = H // N_CORES  # 4 q heads per core
ROWS = B * S  # 4096
RPC = S // N_CORES  # 256 output rows per core per batch

F32 = mybir.dt.float32
BF16 = mybir.dt.bfloat16
EXP = mybir.ActivationFunctionType.Exp
ADD = mybir.AluOpType.add
MULT = mybir.AluOpType.mult
DIV = mybir.AluOpType.divide

QKV = 384  # 256 q | 64 k | 64 v
ROPE_W = 320  # rope applies to q + k
VB = 96  # per-chunk block in the PV weights: 64 V | 32 ones


def build():
    nc = bacc.Bacc("TRN2", target_bir_lowering=False, debug=False, num_devices=N_CORES)

    xt = nc.declare_dram_parameter("xt", [D, ROWS], BF16, isOutput=False)
    wqkv = nc.declare_dram_parameter("wqkv", [D, QKV], BF16, isOutput=False)
    wo = nc.declare_dram_parameter("wo", [D, D], BF16, isOutput=False)
    ropec = nc.declare_dram_parameter("ropec", [S, ROPE_W], BF16, isOutput=False)
    ropes = nc.declare_dram_parameter("ropes", [S, ROPE_W], BF16, isOutput=False)
    maskm = nc.declare_dram_parameter("maskm", [128, 256], BF16, isOutput=False)
    out = nc.declare_dram_parameter("out", [2 * RPC, D], F32, isOutput=True)

    with tile.TileContext(nc) as tc:
        with (
            tc.tile_pool(name="sb", bufs=1) as sb,
            tc.tile_pool(name="ps", bufs=1, space="PSUM") as ps,
            tc.tile_pool(name="dr", bufs=1, space="DRAM") as dr,
        ):
            # ---- constants / weights first so projection starts ASAP ----
            identf = sb.tile([128, 128], F32, tag="identf")
            make_identity(nc, identf[:])
            identb = sb.tile([128, 128], BF16, tag="identb")
            nc.vector.tensor_copy(identb[:], identf[:])
            wqkv_sb = []
            for k in range(16):
                w = sb.tile([128, QKV], BF16, tag=f"wqkv{k}", name=f"wqkv_sb{k}")
                nc.sync.dma_start(out=w[:], in_=wqkv[128 * k : 128 * (k + 1), :])
                wqkv_sb.append(w)
            maskm_sb = sb.tile([128, 256], BF16, tag="maskm")
            # rope tables fully resident in bf16 (loaded just-in-time below)
            ct_all = sb.tile([128, 16 * ROPE_W], BF16, tag="ct_all")
            st_all = sb.tile([128, 16 * ROPE_W], BF16, tag="st_all")
            # wo tiles are declared here but DMA'd one per phase-1 rowblock so
            # the 8 MB doesn't delay the startup x^T loads.
            wo_sb = [
                sb.tile([128, D], BF16, tag=f"wo{k}", name=f"wo_sb{k}")
                for k in range(16)
            ]

            # ---- persistent per-batch activation tiles ----
            qt_sb = [[None, None], [None, None]]  # [b][i]: [128, 2048] bf16
            kt_sb = [None, None]  # [b]: [128, 2048] bf16 (KT replicated 0:64/64:128)
            vones = [None, None]  # [b]: [128, 16*VB] bf16 (V | 32 ones columns)
            for b in range(B):
                for i in range(2):
                    t = sb.tile([128, S], BF16, tag=f"qt{b}{i}", name=f"qt{b}{i}")
                    qt_sb[b][i] = t
                kt_sb[b] = sb.tile([128, S], BF16, tag=f"kt{b}", name=f"kt{b}")
                v = sb.tile([128, 16 * VB], BF16, tag=f"v{b}", name=f"vones{b}")
                nc.vector.memset(v[:], 1.0)
                vones[b] = v

            # ================= phase 1: QKV projection + RoPE + transposes
            pend = []  # lagged transpose work
            for rb in range(8):  # 512-row blocks of the flattened (B*S) rows
                xts = []
                for k in range(16):
                    t = sb.tile([128, 512], BF16, tag="xt", bufs=18, name=f"xt_{rb}_{k}")
                    # first block on the (startup-idle) ACT HWDGE queue so it
                    # streams in parallel with the weight loads on Sync
                    eng = nc.scalar if rb == 0 else nc.sync
                    eng.dma_start(
                        out=t[:],
                        in_=xt[128 * k : 128 * (k + 1), 512 * rb : 512 * (rb + 1)],
                    )
                    xts.append(t)
                # pace the (phase-4) wo loads: two of its 16 row-tiles per block
                if rb == 0:
                    nc.sync.dma_start(out=maskm_sb[:], in_=maskm[:])
                for w in (2 * rb, 2 * rb + 1):
                    nc.sync.dma_start(out=wo_sb[w][:], in_=wo[128 * w : 128 * (w + 1), :])
                if rb < 4:  # rope tiles for this block's positions (b1 reuses them)
                    for kc2 in range(4 * rb, 4 * rb + 4):
                        nc.sync.dma_start(
                            out=ct_all[:, ROPE_W * kc2 : ROPE_W * (kc2 + 1)],
                            in_=ropec[128 * kc2 : 128 * (kc2 + 1), :],
                        )
                        nc.sync.dma_start(
                            out=st_all[:, ROPE_W * kc2 : ROPE_W * (kc2 + 1)],
                            in_=ropes[128 * kc2 : 128 * (kc2 + 1), :],
                        )
                for rt in range(4):
                    r = 4 * rb + rt  # global 128-row tile index (0..31)
                    b = r // 16
                    kc = r % 16  # position tile within the batch
                    pq = ps.tile([128, QKV], F32, tag="pq", bufs=2, name=f"pq_{r}")
                    for k in range(16):
                        nc.tensor.matmul(
                            pq[:],
                            xts[k][:, 128 * rt : 128 * (rt + 1)],
                            wqkv_sb[k][:],
                            start=(k == 0),
                            stop=(k == 15),
                        )
                    # single psum read frees the pq slot in ~0.6us; RoPE and
                    # the V copy then work from SBUF (bf16 fast modes)
                    pqc = sb.tile([128, QKV], BF16, tag="pqc", bufs=2, name=f"pc_{r}")
                    nc.vector.tensor_copy(pqc[:], pq[:])
                    # RoPE over q + k (5 head-blocks of [32r|32i])
                    ct = ct_all[:, ROPE_W * kc : ROPE_W * (kc + 1)]
                    st = st_all[:, ROPE_W * kc : ROPE_W * (kc + 1)]
                    tmp1 = sb.tile([128, ROPE_W], BF16, tag="tmp1", bufs=2, name=f"t1_{r}")
                    tmp2 = sb.tile([128, ROPE_W], BF16, tag="tmp2", bufs=2, name=f"t2_{r}")
                    qk = sb.tile([128, ROPE_W], BF16, tag="qk", bufs=4, name=f"qk_{r}")
                    nc.vector.tensor_tensor(tmp1[:], pqc[:, 0:ROPE_W], ct, op=MULT)
                    pqv = pqc[:, 0:ROPE_W].rearrange("p (h s j) -> p h s j", s=2, j=32)
                    t2v = tmp2[:].rearrange("p (h s j) -> p h s j", s=2, j=32)
                    stv = st.rearrange("p (h s j) -> p h s j", s=2, j=32)
                    # out real-half = q_imag * (-sin); out imag-half = q_real * (+sin)
                    nc.vector.tensor_tensor(
                        t2v[:, :, 0, :], pqv[:, :, 1, :], stv[:, :, 0, :], op=MULT
                    )
                    nc.vector.tensor_tensor(
                        t2v[:, :, 1, :], pqv[:, :, 0, :], stv[:, :, 1, :], op=MULT
                    )
                    nc.vector.tensor_tensor(qk[:], tmp1[:], tmp2[:], op=ADD)
                    # V -> bf16 into the ones-padded PV weights
                    nc.scalar.copy(vones[b][:, VB * kc : VB * kc + 64], pqc[:, 320:384])
                    # PE transposes, lagged one rowtile so the RoPE chain has
                    # a full projection's lead time
                    pend.append((qk, b, kc, r))
                    todo = [pend.pop(0)] if len(pend) > 1 else []
                    if r == 31:
                        todo += [pend.pop(0)]
                    for tqk, tb, tkc, tr in todo:
                        for i in range(2):
                            tp = ps.tile(
                                [128, 128], BF16, tag="pq", bufs=2, name=f"tp_{tr}_{i}"
                            )
                            nc.tensor.transpose(
                                tp[:], tqk[:, 128 * i : 128 * (i + 1)], identb[:]
                            )
                            nc.vector.tensor_copy(
                                qt_sb[tb][i][:, 128 * tkc : 128 * (tkc + 1)], tp[:]
                            )
                        # K: [128, 64] -> [64, 128], then copy into both
                        # partition halves of kt (scores read either replica)
                        tpk = ps.tile([64, 128], BF16, tag="pq", bufs=2, name=f"tpk_{tr}")
                        nc.tensor.transpose(tpk[:], tqk[:, 256:320], identb[:])
                        nc.vector.tensor_copy(
                            kt_sb[tb][0:64, 128 * tkc : 128 * (tkc + 1)], tpk[:]
                        )
                        nc.vector.tensor_copy(
                            kt_sb[tb][64:128, 128 * tkc : 128 * (tkc + 1)], tpk[:]
                        )

            # ================= phases 2+3: attention per batch, then AllToAll
            # run attention strictly after phase 1 (the overlap costs more in
            # in-order-queue stalls than it saves)
            a2a_out = [[None, None], [None, None]]  # [b][half]
            last_pv = None  # ordering handle for the output projection
            ats = [
                sb.tile([128, 512], BF16, tag=f"at{k}", name=f"at_{k}") for k in range(16)
            ]
            for b in range(B):
                a2a_in = [
                    dr.tile([1024, RPC], BF16, tag=f"a2ai{b}{p}", name=f"a2a_in{b}{p}")
                    for p in range(2)
                ]
                a2a_out[b] = [
                    dr.tile([1024, RPC], BF16, tag=f"a2ao{b}{p}", name=f"a2a_out{b}{p}")
                    for p in range(2)
                ]
                for hp in range(2):  # head pair (2hp, 2hp+1): one PE row-tile each
                    qtile = qt_sb[b][hp]
                    for qc in range(4):  # 512-wide q chunks
                        ots = [
                            ps.tile([96, 512], F32, tag=f"ot{hi}", bufs=1,
                                    name=f"ot_{b}_{hp}_{qc}_{hi}")
                            for hi in range(2)
                        ]
                        # kpos chunk groups: two packed diagonal groups first
                        # (columns < the chunk's causal start are dropped), then
                        # clean pairs descending. Each chunk is (kch, colstart,
                        # width): scores/exp/PV only touch cols [cs, 512).
                        groups = [
                            [(4 * qc, 0, 512), (4 * qc + 1, 128, 384)],
                            [(4 * qc + 2, 256, 256), (4 * qc + 3, 384, 128)],
                        ] + [
                            [(2 * p, 0, 512), (2 * p + 1, 0, 512)]
                            for p in reversed(range(2 * qc))
                        ]
                        pend_pv = None
                        for gi, chunks in enumerate(groups + [None]):
                            pts = []
                            if chunks is not None:
                                diag = gi < 2
                                for hi in range(2):
                                    qrow = 64 * hi
                                    sp = ps.tile(
                                        [128, 1024], F32, tag="s", bufs=2,
                                        name=f"s_{b}_{hp}_{qc}_{gi}_{hi}",
                                    )
                                    off, offs = 0, []
                                    for kch, cs, w in chunks:
                                        nc.tensor.matmul(
                                            sp[:, off : off + w],
                                            kt_sb[b][qrow : qrow + 64, 128 * kch : 128 * (kch + 1)],
                                            qtile[qrow : qrow + 64, 512 * qc + cs : 512 * qc + cs + w],
                                            start=True,
                                            stop=True,
                                        )
                                        offs.append(off)
                                        off += w
                                    pt = sb.tile(
                                        [128, 1024], BF16, tag="pt", bufs=6,
                                        name=f"pt_{b}_{hp}_{qc}_{gi}_{hi}",
                                    )
                                    nc.scalar.activation(pt[:, 0:off], sp[:, 0:off], EXP, scale=0.125)
                                    if diag:
                                        # per-chunk leading 128 cols are the causal
                                        # triangle: one strided 0/1 multiply covers
                                        # both chunks of the group
                                        stride = 512 if gi == 0 else 256
                                        ptv = pt[:, 0 : 2 * stride].rearrange(
                                            "p (g c) -> p g c", c=stride
                                        )[:, :, 0:128]
                                        mkv = maskm_sb[:].rearrange("p (g c) -> p g c", c=128)
                                        nc.vector.tensor_tensor(ptv, ptv, mkv, op=MULT)
                                    pts.append((pt, chunks, offs))
                            if pend_pv is not None:
                                for hi in range(2):
                                    pt, chunks2, offs2 = pend_pv[hi]
                                    for (kch, cs, w), po in zip(chunks2, offs2):
                                        mm = nc.tensor.matmul(
                                            ots[hi][:, cs : cs + w],
                                            vones[b][:, VB * kch : VB * kch + VB],
                                            pt[:, po : po + w],
                                            start=(kch == 4 * qc),
                                            stop=(kch == (1 if qc else 3)),
                                            skip_group_check=True,
                                        )
                                        last_pv = mm
                            pend_pv = pts if chunks is not None else None
                        # normalize: rows 0:64 are V^T P, rows 64:96 the softmax
                        # sums (32 identical copies) -> 32-lane reciprocal + two
                        # 32-partition multiplies, all on DVE
                        for hi in range(2):
                            h = 2 * hp + hi
                            ot = ots[hi]
                            sums = sb.tile([32, 512], F32, tag="sums", bufs=2, name=f"sm_{b}_{h}_{qc}")
                            nc.vector.tensor_copy(sums[:], ot[64:96, :])
                            inv = sb.tile([32, 512], F32, tag="inv", bufs=2, name=f"iv_{b}_{h}_{qc}")
                            nc.vector.reciprocal_approx_fast(inv[:], sums[:])
                            osb = sb.tile([64, 512], BF16, tag="osb", bufs=3, name=f"o_{b}_{h}_{qc}")
                            nc.vector.tensor_tensor(osb[0:32, :], ot[0:32, :], inv[:], op=MULT)
                            nc.vector.tensor_tensor(osb[32:64, :], ot[32:64, :], inv[:], op=MULT)
                            # stage into AllToAll layout: dest j rows 128j..128j+128
                            # in the lo (heads 0-1) / hi (heads 2-3) half buffer
                            for half in range(2):
                                j = 2 * qc + half
                                nc.sync.dma_start(
                                    out=a2a_in[hp][128 * j + 64 * hi : 128 * j + 64 * (hi + 1), :],
                                    in_=osb[:, 256 * half : 256 * (half + 1)],
                                )
                    # lo half-collective after heads 0-1, hi after heads 2-3
                    nc.gpsimd.collective_compute(
                        "AllToAll",
                        mybir.AluOpType.bypass,
                        replica_groups=[list(range(N_CORES))],
                        ins=[a2a_in[hp][:].opt()],
                        outs=[a2a_out[b][hp][:].opt()],
                    )
                    # at-tile loads right after each half-collective, on the
                    # gpsimd queue (so Sync/PE never block on a collective);
                    # the last batch's hi loads split with the idle ACT
                    # queue to halve the issue tail gating oproj phase B.
                    # Even k-tiles come from lo, odd from hi.
                    for k in range(hp, 16, 2):
                        eng = nc.scalar if (b == 1 and hp == 1 and k % 4 == 3) else nc.gpsimd
                        eng.dma_start(
                            out=ats[k][:, 256 * b : 256 * (b + 1)],
                            in_=a2a_out[b][hp][128 * (k // 2) : 128 * (k // 2) + 128, :],
                        )

            # ================= phase 4: output projection (my 512 rows @ wo)
            # phase A: batch-0 rows (need only batch-0 collectives); phase B:
            # batch-1. Even k-tiles first: they arrive with the lo half.
            # Explicit deps pin the in-order PE queue to [attn b1][A][B].
            korder = list(range(0, 16, 2)) + list(range(1, 16, 2))
            prev_phase_last = last_pv
            for rows in ([0, 1], [2, 3]):
                phase_last = None
                for n in range(4):
                    for row in rows:
                        op = ps.tile([128, 512], F32, tag="pq", bufs=2, name=f"op_{n}_{row}")
                        for ki, k in enumerate(korder):
                            mm = nc.tensor.matmul(
                                op[:],
                                ats[k][:, 128 * row : 128 * (row + 1)],
                                wo_sb[k][:, 512 * n : 512 * (n + 1)],
                                start=(ki == 0),
                                stop=(ki == 15),
                            )
                            if ki == 0 and prev_phase_last is not None:
                                add_dep_helper(
                                    mm.ins,
                                    prev_phase_last.ins,
                                    sync=False,
                                    reason="pin oproj phase order in PE queue",
                                )
                            phase_last = mm
                        ob = sb.tile([128, 512], F32, tag="outsb", bufs=2, name=f"ob_{n}_{row}")
                        nc.vector.tensor_copy(ob[:], op[:])
                        nc.sync.dma_start(
                            out=out[128 * row : 128 * (row + 1), 512 * n : 512 * (n + 1)],
                            in_=ob[:],
                        )
                prev_phase_last = phase_last

    nc.finalize()
    return nc


_NC_CACHE = None


def _get_nc():
    global _NC_CACHE
    if _NC_CACHE is None:
        _NC_CACHE = build()
    return _NC_CACHE


def _prep_inputs(x, freqs_cis, mask, wq, wk, wv, wo):
    """Host-side sharding / layout prep. Returns per-core input maps."""
    bf16 = ml_dtypes.bfloat16
    xt = np.ascontiguousarray(x.reshape(ROWS, D).T.astype(bf16))  # [D, B*S]
    cos = np.ascontiguousarray(freqs_cis[:, :, 0])  # [S, 32]
    sin = np.ascontiguousarray(freqs_cis[:, :, 1])
    c64 = np.concatenate([cos, cos], axis=1)  # [S, 64]
    s64 = np.concatenate([-sin, sin], axis=1)
    ropec = np.ascontiguousarray(np.tile(c64, (1, 5)).astype(bf16))  # [S, 320]
    ropes = np.ascontiguousarray(np.tile(s64, (1, 5)).astype(bf16))
    # causal 0/1 keep-triangle (keep iff q-col >= kpos-row), replicated twice so
    # one strided multiply masks both chunks of a diagonal group. Derived from
    # the mask input: maskm[r, c] = keep(mask[c, r]) for the leading 128x128.
    tri = (mask[0:128, 0:128].T > -1.0).astype(bf16)
    maskm = np.ascontiguousarray(np.concatenate([tri, tri], axis=1))
    perm = np.concatenate([np.arange(0, 64, 2), np.arange(1, 64, 2)])  # de-interleave
    wo_c = np.ascontiguousarray(wo.astype(bf16))

    in_maps = []
    for c in range(N_CORES):
        heads = range(HPC * c, HPC * (c + 1))
        kv = c // 2
        wq_c = np.concatenate([wq[:, 64 * h + perm] for h in heads], axis=1)
        wk_c = wk[:, 64 * kv + perm]
        wv_c = wv[:, 64 * kv : 64 * (kv + 1)]
        wqkv_c = np.ascontiguousarray(
            np.concatenate([wq_c, wk_c, wv_c], axis=1).astype(bf16)
        )
        in_maps.append(
            {
                "xt": xt,
                "wqkv": wqkv_c,
                "wo": wo_c,
                "ropec": ropec,
                "ropes": ropes,
                "maskm": maskm,
            }
        )
    return in_maps


def kernel(x, freqs_cis, mask, wq, wk, wv, wo, _trace=False, _trace_kwargs=None):
    nc = _get_nc()
    in_maps = _prep_inputs(
        np.asarray(x, np.float32),
        np.asarray(freqs_cis, np.float32),
        np.asarray(mask, np.float32),
        np.asarray(wq, np.float32),
        np.asarray(wk, np.float32),
        np.asarray(wv, np.float32),
        np.asarray(wo, np.float32),
    )
    kwargs = {}
    if _trace:
        kwargs["trace"] = True
        if _trace_kwargs:
            kwargs.update(_trace_kwargs)
    res = run_bass_kernel_spmd(nc, in_maps, core_ids=list(range(N_CORES)), **kwargs)
    full = np.empty((B, S, D), np.float32)
    for c in range(N_CORES):
        oc = res.results[c]["out"]
        full[0, RPC * c : RPC * (c + 1)] = oc[0:RPC]
        full[1, RPC * c : RPC * (c + 1)] = oc[RPC : 2 * RPC]
    if _trace:
        kernel.last_results = res
    return full


if __name__ == "__main__":
    print("building...")
    nc = _get_nc()
    print("built")


# revision 17
# speedup vs baseline: 1.4030x; 1.1028x over previous
"""Distributed GQA attention kernel for 8 TRN2 NeuronCores.

Problem: B=2, S=2048, D=2048, H=32 heads, KVH=4 kv-heads, HD=64 (GQA),
RoPE + causal attention + output projection, fp32 inputs/outputs.

Sharding: tensor-parallel over heads. Core c owns q-heads [4c..4c+4) and
kv-head c//2 (each kv head is shared by 2 cores; its tiny K/V projection is
recomputed on both). Per core:
  1. QKV projection from the replicated, host-pre-transposed x^T (bf16) with
     the core's weight column slice packed as one [2048, 384] bf16 rhs
     (256 q | 64 k | 64 v).
  2. RoPE in natural layout on the DVE (weight columns de-interleaved on host
     so each head is [32 reals | 32 imags]; q.k is invariant under a common
     permutation of head dims).
  3. Q,K transposed on the PE; K's [64,128] transpose is copied to both
     partition halves of kt so either 64-row replica feeds the scores matmul.
     Scores are computed transposed (scoresT[kpos, q]) so the softmax
     normalizer falls out of ones-columns appended to V in the PV matmul.
  4. Causal flash attention in bf16, kpos chunks processed in pairs: two
     128-kpos score matmuls into one 2-bank psum, one [128,1024] exp on ACT,
     multiplicative 0/1 mask on the diagonal chunks (post-exp, bf16 DVE),
     two PV matmuls. Diagonal pairs run FIRST within each q chunk so the
     DVE mask latency hides behind the clean chunks; phase-1 transposes lag
     their RoPE by one row tile for the same reason.
  5. Normalization entirely on DVE: 32 replicated ones-columns in the PV
     weights land 32 identical sum rows in psum partitions 64:96; one 32-lane
     fast reciprocal + two 32-partition multiplies normalize the output.
     (gpsimd carries ONLY collectives + at-tile loads, so batch-1 compute
     never queues behind the batch-0 AllToAll.)
  6. Attention outputs staged (transposed) to DRAM in AllToAll layout; TWO
     half-collectives per batch (heads 0-1, then heads 2-3) so comm starts
     halfway through each batch's attention and the final collective only
     carries 0.5 MB.
  7. Row-sharded output projection (rows [256c..256c+256) of each batch)
     against the fully-resident bf16 wo, even k-tiles first (they arrive
     with the lo half-collective), with explicit ordering deps so the
     in-order PE queue never waits on a later collective before running work
     that is already eligible.
Host gathers the 8 [512, 2048] row-slices into the (2, 2048, 2048) output.
"""

import os
import sys

sys.path.insert(0, "/opt/trn_rl_repo")

import ml_dtypes
import numpy as np

import concourse.bass as bass
import concourse.mybir as mybir
import concourse.tile as tile
from concourse import bacc
from concourse.bass_utils import run_bass_kernel_spmd
from concourse.masks import make_identity
from concourse.tile_rust import add_dep_helper

N_CORES = 8
B, S, D = 2, 2048, 2048
H, KVH, HD = 32, 4, 64
HPC = H // N_CORES  # 4 q heads per core
ROWS = B * S  # 4096
RPC = S // N_CORES  # 256 output rows per core per batch

F32 = mybir.dt.float32
BF16 = mybir.dt.bfloat16
EXP = mybir.ActivationFunctionType.Exp
ADD = mybir.AluOpType.add
MULT = mybir.AluOpType.mult
DIV = mybir.AluOpType.divide

QKV = 384  # 256 q | 64 k | 64 v
ROPE_W = 320  # rope applies to q + k
VB = 128  # per-chunk block in the PV weights: 64 V | 64 ones


def build():
    nc = bacc.Bacc("TRN2", target_bir_lowering=False, debug=False, num_devices=N_CORES)

    xt = nc.declare_dram_parameter("xt", [D, ROWS], BF16, isOutput=False)
    wqkv = nc.declare_dram_parameter("wqkv", [D, QKV], BF16, isOutput=False)
    wo = nc.declare_dram_parameter("wo", [D, D], BF16, isOutput=False)
    ropec = nc.declare_dram_parameter("ropec", [S, ROPE_W], BF16, isOutput=False)
    ropes = nc.declare_dram_parameter("ropes", [S, ROPE_W], BF16, isOutput=False)
    maskm = nc.declare_dram_parameter("maskm", [128, 256], BF16, isOutput=False)
    out = nc.declare_dram_parameter("out", [2 * RPC, D], F32, isOutput=True)

    with tile.TileContext(nc) as tc:
        with (
            tc.tile_pool(name="sb", bufs=1) as sb,
            tc.tile_pool(name="ps", bufs=1, space="PSUM") as ps,
            tc.tile_pool(name="dr", bufs=1, space="DRAM") as dr,
        ):
            # ---- constants / weights first so projection starts ASAP ----
            identf = sb.tile([128, 128], F32, tag="identf")
            make_identity(nc, identf[:])
            identb = sb.tile([128, 128], BF16, tag="identb")
            nc.vector.tensor_copy(identb[:], identf[:])
            wqkv_sb = []
            for k in range(16):
                w = sb.tile([128, QKV], BF16, tag=f"wqkv{k}", name=f"wqkv_sb{k}")
                nc.sync.dma_start(out=w[:], in_=wqkv[128 * k : 128 * (k + 1), :])
                wqkv_sb.append(w)
            maskm_sb = sb.tile([128, 256], BF16, tag="maskm")
            # rope tables fully resident in bf16 (loaded just-in-time below)
            ct_all = sb.tile([128, 16 * ROPE_W], BF16, tag="ct_all")
            st_all = sb.tile([128, 16 * ROPE_W], BF16, tag="st_all")
            # wo tiles are declared here but DMA'd one per phase-1 rowblock so
            # the 8 MB doesn't delay the startup x^T loads.
            wo_sb = [
                sb.tile([128, D], BF16, tag=f"wo{k}", name=f"wo_sb{k}")
                for k in range(16)
            ]

            # ---- persistent per-batch activation tiles ----
            # ktz[b][hi] holds K^T on partition half hi and ZEROS on the other
            # half, so the scores matmul can contract over all 128 partitions
            # (the other head's q rows hit the zero half): every PE instruction
            # stays in 128x128 tile mode - no mode-switch drains.
            qt_sb = [[None, None], [None, None]]  # [b][i]: [128, 2048] bf16
            ktz = [[None, None], [None, None]]  # [b][hi]
            vones = [None, None]  # [b]: [128, 16*VB] bf16 (V | 64 ones columns)
            for b in range(B):
                for i in range(2):
                    t = sb.tile([128, S], BF16, tag=f"qt{b}{i}", name=f"qt{b}{i}")
                    qt_sb[b][i] = t
                    kz = sb.tile([128, S], BF16, tag=f"ktz{b}{i}", name=f"ktz{b}{i}")
                    nc.vector.memset(kz[:], 0.0)
                    ktz[b][i] = kz
                v = sb.tile([128, 16 * VB], BF16, tag=f"v{b}", name=f"vones{b}")
                nc.vector.memset(v[:], 1.0)
                vones[b] = v

            # ================= phase 1: QKV projection + RoPE + transposes
            pend = []  # lagged transpose work
            for rb in range(8):  # 512-row blocks of the flattened (B*S) rows
                xts = []
                for k in range(16):
                    t = sb.tile([128, 512], BF16, tag="xt", bufs=18, name=f"xt_{rb}_{k}")
                    # first block on the (startup-idle) ACT HWDGE queue so it
                    # streams in parallel with the weight loads on Sync
                    eng = nc.scalar if rb == 0 else nc.sync
                    eng.dma_start(
                        out=t[:],
                        in_=xt[128 * k : 128 * (k + 1), 512 * rb : 512 * (rb + 1)],
                    )
                    xts.append(t)
                # pace the (phase-4) wo loads: two of its 16 row-tiles per block
                if rb == 0:
                    nc.sync.dma_start(out=maskm_sb[:], in_=maskm[:])
                for w in (2 * rb, 2 * rb + 1):
                    nc.sync.dma_start(out=wo_sb[w][:], in_=wo[128 * w : 128 * (w + 1), :])
                if rb < 4:  # rope tiles for this block's positions (b1 reuses them)
                    for kc2 in range(4 * rb, 4 * rb + 4):
                        nc.sync.dma_start(
                            out=ct_all[:, ROPE_W * kc2 : ROPE_W * (kc2 + 1)],
                            in_=ropec[128 * kc2 : 128 * (kc2 + 1), :],
                        )
                        nc.sync.dma_start(
                            out=st_all[:, ROPE_W * kc2 : ROPE_W * (kc2 + 1)],
                            in_=ropes[128 * kc2 : 128 * (kc2 + 1), :],
                        )
                for rt in range(4):
                    r = 4 * rb + rt  # global 128-row tile index (0..31)
                    b = r // 16
                    kc = r % 16  # position tile within the batch
                    pq = ps.tile([128, QKV], F32, tag="pq", bufs=2, name=f"pq_{r}")
                    for k in range(16):
                        nc.tensor.matmul(
                            pq[:],
                            xts[k][:, 128 * rt : 128 * (rt + 1)],
                            wqkv_sb[k][:],
                            start=(k == 0),
                            stop=(k == 15),
                        )
                    # single psum read frees the pq slot in ~0.6us; RoPE and
                    # the V copy then work from SBUF (bf16 fast modes)
                    pqc = sb.tile([128, QKV], BF16, tag="pqc", bufs=2, name=f"pc_{r}")
                    nc.vector.tensor_copy(pqc[:], pq[:])
                    # RoPE over q + k (5 head-blocks of [32r|32i])
                    ct = ct_all[:, ROPE_W * kc : ROPE_W * (kc + 1)]
                    st = st_all[:, ROPE_W * kc : ROPE_W * (kc + 1)]
                    tmp1 = sb.tile([128, ROPE_W], BF16, tag="tmp1", bufs=2, name=f"t1_{r}")
                    tmp2 = sb.tile([128, ROPE_W], BF16, tag="tmp2", bufs=2, name=f"t2_{r}")
                    qk = sb.tile([128, ROPE_W], BF16, tag="qk", bufs=4, name=f"qk_{r}")
                    nc.vector.tensor_tensor(tmp1[:], pqc[:, 0:ROPE_W], ct, op=MULT)
                    pqv = pqc[:, 0:ROPE_W].rearrange("p (h s j) -> p h s j", s=2, j=32)
                    t2v = tmp2[:].rearrange("p (h s j) -> p h s j", s=2, j=32)
                    stv = st.rearrange("p (h s j) -> p h s j", s=2, j=32)
                    # out real-half = q_imag * (-sin); out imag-half = q_real * (+sin)
                    nc.vector.tensor_tensor(
                        t2v[:, :, 0, :], pqv[:, :, 1, :], stv[:, :, 0, :], op=MULT
                    )
                    nc.vector.tensor_tensor(
                        t2v[:, :, 1, :], pqv[:, :, 0, :], stv[:, :, 1, :], op=MULT
                    )
                    nc.vector.tensor_tensor(qk[:], tmp1[:], tmp2[:], op=ADD)
                    # V -> bf16 into the ones-padded PV weights
                    nc.scalar.copy(vones[b][:, VB * kc : VB * kc + 64], pqc[:, 320:384])
                    # PE transposes, lagged one rowtile so the RoPE chain has
                    # a full projection's lead time
                    pend.append((qk, b, kc, r))
                    todo = [pend.pop(0)] if len(pend) > 1 else []
                    if r == 31:
                        todo += [pend.pop(0)]
                    for tqk, tb, tkc, tr in todo:
                        for i in range(2):
                            tp = ps.tile(
                                [128, 128], BF16, tag="pq", bufs=2, name=f"tp_{tr}_{i}"
                            )
                            nc.tensor.transpose(
                                tp[:], tqk[:, 128 * i : 128 * (i + 1)], identb[:]
                            )
                            nc.vector.tensor_copy(
                                qt_sb[tb][i][:, 128 * tkc : 128 * (tkc + 1)], tp[:]
                            )
                        # K: [128, 64] -> [64, 128], then copy into the live
                        # half of each zero-padded kt variant
                        tpk = ps.tile([64, 128], BF16, tag="pq", bufs=2, name=f"tpk_{tr}")
                        nc.tensor.transpose(tpk[:], tqk[:, 256:320], identb[:])
                        nc.vector.tensor_copy(
                            ktz[tb][0][0:64, 128 * tkc : 128 * (tkc + 1)], tpk[:]
                        )
                        nc.vector.tensor_copy(
                            ktz[tb][1][64:128, 128 * tkc : 128 * (tkc + 1)], tpk[:]
                        )

            # ================= phases 2+3: attention per batch, then AllToAll
            # run attention strictly after phase 1 (the overlap costs more in
            # in-order-queue stalls than it saves)
            a2a_out = [[None, None], [None, None]]  # [b][half]
            last_pv = None  # ordering handles for the output projection
            pin_a = None  # oproj phase A starts after batch-1's first head-pair
            ats = [
                sb.tile([128, 512], BF16, tag=f"at{k}", name=f"at_{k}") for k in range(16)
            ]
            for b in range(B):
                a2a_in = [
                    dr.tile([1024, RPC], BF16, tag=f"a2ai{b}{p}", name=f"a2a_in{b}{p}")
                    for p in range(2)
                ]
                a2a_out[b] = [
                    dr.tile([1024, RPC], BF16, tag=f"a2ao{b}{p}", name=f"a2a_out{b}{p}")
                    for p in range(2)
                ]
                for hp in range(2):  # head pair (2hp, 2hp+1): one PE row-tile each
                    qtile = qt_sb[b][hp]
                    for qc in range(4):  # 512-wide q chunks
                        ots = [
                            ps.tile([128, 512], F32, tag=f"ot{hi}", bufs=1,
                                    name=f"ot_{b}_{hp}_{qc}_{hi}")
                            for hi in range(2)
                        ]
                        # kpos chunk groups: two packed diagonal groups first
                        # (columns < the chunk's causal start are dropped), then
                        # clean pairs descending. Each chunk is (kch, colstart,
                        # width): scores/exp/PV only touch cols [cs, 512).
                        groups = [
                            [(4 * qc, 0, 512), (4 * qc + 1, 128, 384)],
                            [(4 * qc + 2, 256, 256), (4 * qc + 3, 384, 128)],
                        ] + [
                            [(2 * p, 0, 512), (2 * p + 1, 0, 512)]
                            for p in reversed(range(2 * qc))
                        ]
                        pend_pv = None
                        for gi, chunks in enumerate(groups + [None]):
                            pts = []
                            if chunks is not None:
                                diag = gi < 2
                                for hi in range(2):
                                    sp = ps.tile(
                                        [128, 1024], F32, tag="s", bufs=2,
                                        name=f"s_{b}_{hp}_{qc}_{gi}_{hi}",
                                    )
                                    off, offs = 0, []
                                    for kch, cs, w in chunks:
                                        nc.tensor.matmul(
                                            sp[:, off : off + w],
                                            ktz[b][hi][:, 128 * kch : 128 * (kch + 1)],
                                            qtile[:, 512 * qc + cs : 512 * qc + cs + w],
                                            start=True,
                                            stop=True,
                                        )
                                        offs.append(off)
                                        off += w
                                    pt = sb.tile(
                                        [128, 1024], BF16, tag="pt", bufs=6,
                                        name=f"pt_{b}_{hp}_{qc}_{gi}_{hi}",
                                    )
                                    nc.scalar.activation(pt[:, 0:off], sp[:, 0:off], EXP, scale=0.125)
                                    if diag:
                                        # per-chunk leading 128 cols are the causal
                                        # triangle: one strided 0/1 multiply covers
                                        # both chunks of the group
                                        stride = 512 if gi == 0 else 256
                                        ptv = pt[:, 0 : 2 * stride].rearrange(
                                            "p (g c) -> p g c", c=stride
                                        )[:, :, 0:128]
                                        mkv = maskm_sb[:].rearrange("p (g c) -> p g c", c=128)
                                        nc.vector.tensor_tensor(ptv, ptv, mkv, op=MULT)
                                    pts.append((pt, chunks, offs))
                            if pend_pv is not None:
                                for hi in range(2):
                                    pt, chunks2, offs2 = pend_pv[hi]
                                    for (kch, cs, w), po in zip(chunks2, offs2):
                                        mm = nc.tensor.matmul(
                                            ots[hi][:, cs : cs + w],
                                            vones[b][:, VB * kch : VB * kch + VB],
                                            pt[:, po : po + w],
                                            start=(kch == 4 * qc),
                                            stop=(kch == (1 if qc else 3)),
                                            skip_group_check=True,
                                        )
                                        last_pv = mm
                            pend_pv = pts if chunks is not None else None
                        # normalize: rows 0:64 are V^T P, rows 64:128 the softmax
                        # sums (64 identical copies): copy sums to base-0 SBUF
                        # (custom-DVE recip needs aligned bases), reciprocal,
                        # one 64-lane multiply straight from psum.
                        for hi in range(2):
                            h = 2 * hp + hi
                            ot = ots[hi]
                            sums = sb.tile([64, 512], F32, tag="sums", bufs=2, name=f"sm_{b}_{h}_{qc}")
                            nc.vector.tensor_copy(sums[:], ot[64:128, :])
                            inv = sb.tile([64, 512], F32, tag="inv", bufs=2, name=f"iv_{b}_{h}_{qc}")
                            nc.vector.reciprocal_approx_fast(inv[:], sums[:])
                            osb = sb.tile([64, 512], BF16, tag="osb", bufs=3, name=f"o_{b}_{h}_{qc}")
                            nc.vector.tensor_tensor(osb[:], ot[0:64, :], inv[:], op=MULT)
                            # stage into AllToAll layout: dest j rows 128j..128j+128
                            # in the lo (heads 0-1) / hi (heads 2-3) half buffer
                            for half in range(2):
                                j = 2 * qc + half
                                nc.sync.dma_start(
                                    out=a2a_in[hp][128 * j + 64 * hi : 128 * j + 64 * (hi + 1), :],
                                    in_=osb[:, 256 * half : 256 * (half + 1)],
                                )
                    # lo half-collective after heads 0-1, hi after heads 2-3
                    nc.gpsimd.collective_compute(
                        "AllToAll",
                        mybir.AluOpType.bypass,
                        replica_groups=[list(range(N_CORES))],
                        ins=[a2a_in[hp][:].opt()],
                        outs=[a2a_out[b][hp][:].opt()],
                    )
                    # at-tile loads right after each half-collective, on the
                    # gpsimd queue (so Sync/PE never block on a collective);
                    # the last batch's hi loads split with the idle ACT
                    # queue to halve the issue tail gating oproj phase B.
                    # Even k-tiles come from lo, odd from hi.
                    for k in range(hp, 16, 2):
                        eng = nc.scalar if (b == 1 and hp == 1 and k % 4 == 3) else nc.gpsimd
                        eng.dma_start(
                            out=ats[k][:, 256 * b : 256 * (b + 1)],
                            in_=a2a_out[b][hp][128 * (k // 2) : 128 * (k // 2) + 128, :],
                        )
                    if b == 1 and hp == 0:
                        pin_a = last_pv

            # ================= phase 4: output projection (my 512 rows @ wo)
            # phase A: batch-0 rows (need only batch-0 collectives); phase B:
            # batch-1. Even k-tiles first: they arrive with the lo half.
            # Phase A is pinned only after batch-1's FIRST head-pair, so its
            # matmuls fill the PE slack of the ACT-bound second half; phase B
            # is pinned after A.
            korder = list(range(0, 16, 2)) + list(range(1, 16, 2))
            prev_phase_last = pin_a if pin_a is not None else last_pv
            for rows in ([0, 1], [2, 3]):
                phase_last = None
                for n in range(4):
                    for row in rows:
                        op = ps.tile([128, 512], F32, tag="pq", bufs=2, name=f"op_{n}_{row}")
                        for ki, k in enumerate(korder):
                            mm = nc.tensor.matmul(
                                op[:],
                                ats[k][:, 128 * row : 128 * (row + 1)],
                                wo_sb[k][:, 512 * n : 512 * (n + 1)],
                                start=(ki == 0),
                                stop=(ki == 15),
                            )
                            if ki == 0 and prev_phase_last is not None:
                                add_dep_helper(
                                    mm.ins,
                                    prev_phase_last.ins,
                                    sync=False,
                                    reason="pin oproj phase order in PE queue",
                                )
                            phase_last = mm
                        ob = sb.tile([128, 512], F32, tag="outsb", bufs=2, name=f"ob_{n}_{row}")
                        nc.vector.tensor_copy(ob[:], op[:])
                        nc.sync.dma_start(
                            out=out[128 * row : 128 * (row + 1), 512 * n : 512 * (n + 1)],
                            in_=ob[:],
                        )
                prev_phase_last = phase_last

    nc.finalize()
    return nc


_NC_CACHE = None


def _get_nc():
    global _NC_CACHE
    if _NC_CACHE is None:
        _NC_CACHE = build()
    return _NC_CACHE


def _prep_inputs(x, freqs_cis, mask, wq, wk, wv, wo):
    """Host-side sharding / layout prep. Returns per-core input maps."""
    bf16 = ml_dtypes.bfloat16
    xt = np.ascontiguousarray(x.reshape(ROWS, D).T.astype(bf16))  # [D, B*S]
    cos = np.ascontiguousarray(freqs_cis[:, :, 0])  # [S, 32]
    sin = np.ascontiguousarray(freqs_cis[:, :, 1])
    c64 = np.concatenate([cos, cos], axis=1)  # [S, 64]
    s64 = np.concatenate([-sin, sin], axis=1)
    ropec = np.ascontiguousarray(np.tile(c64, (1, 5)).astype(bf16))  # [S, 320]
    ropes = np.ascontiguousarray(np.tile(s64, (1, 5)).astype(bf16))
    # causal 0/1 keep-triangle (keep iff q-col >= kpos-row), replicated twice so
    # one strided multiply masks both chunks of a diagonal group. Derived from
    # the mask input: maskm[r, c] = keep(mask[c, r]) for the leading 128x128.
    tri = (mask[0:128, 0:128].T > -1.0).astype(bf16)
    maskm = np.ascontiguousarray(np.concatenate([tri, tri], axis=1))
    perm = np.concatenate([np.arange(0, 64, 2), np.arange(1, 64, 2)])  # de-interleave
    wo_c = np.ascontiguousarray(wo.astype(bf16))

    in_maps = []
    for c in range(N_CORES):
        heads = range(HPC * c, HPC * (c + 1))
        kv = c // 2
        wq_c = np.concatenate([wq[:, 64 * h + perm] for h in heads], axis=1)
        wk_c = wk[:, 64 * kv + perm]
        wv_c = wv[:, 64 * kv : 64 * (kv + 1)]
        wqkv_c = np.ascontiguousarray(
            np.concatenate([wq_c, wk_c, wv_c], axis=1).astype(bf16)
        )
        in_maps.append(
            {
                "xt": xt,
                "wqkv": wqkv_c,
                "wo": wo_c,
                "ropec": ropec,
                "ropes": ropes,
                "maskm": maskm,
            }
        )
    return in_maps


def kernel(x, freqs_cis, mask, wq, wk, wv, wo, _trace=False, _trace_kwargs=None):
    nc = _get_nc()
    in_maps = _prep_inputs(
        np.asarray(x, np.float32),
        np.asarray(freqs_cis, np.float32),
        np.asarray(mask, np.float32),
        np.asarray(wq, np.float32),
        np.asarray(wk, np.float32),
        np.asarray(wv, np.float32),
        np.asarray(wo, np.float32),
    )
    kwargs = {}
    if _trace:
        kwargs["trace"] = True
        if _trace_kwargs:
            kwargs.update(_trace_kwargs)
    res = run_bass_kernel_spmd(nc, in_maps, core_ids=list(range(N_CORES)), **kwargs)
    full = np.empty((B, S, D), np.float32)
    for c in range(N_CORES):
        oc = res.results[c]["out"]
        full[0, RPC * c : RPC * (c + 1)] = oc[0:RPC]
        full[1, RPC * c : RPC * (c + 1)] = oc[RPC : 2 * RPC]
    if _trace:
        kernel.last_results = res
    return full


if __name__ == "__main__":
    print("building...")
    nc = _get_nc()
    print("built")
